# revision 28
# baseline (speedup 1.0000x reference)
"""GATv2 (2-layer, heads=1) on 8 trn2 NeuronCores.

Strategy (dst-sharded, degree-class scheduled, pair-packed gathers):
  - Nodes partitioned across 8 cores by dst id (6250 each). Edges (with self
    loops) grouped by dst; each dst padded to a degree-class K (multiple of 4).
    Per class: tiles of m=floor(128/K) dsts x K slots = tile_e edge slots.
  - Layer-1 logit:  e = att.leaky_relu(xl[src]+xr[dst])
                    = 0.2*(p_j+q_i) + 0.8*(relu-sum over sign-sorted, |att|-
                      scaled feature columns), with p=att.xl, q=att.xr.
    Softmax max-subtraction is skipped (shift-invariant; exponents are small).
  - h (layer-1 output) is never materialized: layer 2 only needs
    xl2 = h@Wl2, xr2 = h@Wr2, which are linear in the layer-1 aggregates of
    y = xl@Wl2, z = xl@Wr2 (2 cols each).
  - Per-edge gather: ONE 512B descriptor per edge slot fetches a PAIR of
    256B table rows (pair idx = row>>1 fits int16); the wanted half is
    selected on-chip with 3 DVE bitwise ops and a host-shipped parity mask.
  - Per-dst aggregation via small per-tile matmuls with a constant block
    one-hot scaled by s=exp(e).
  - Layer-2 node table is compact bf16 (16B/row): tiny AllGather (0.8MB),
    then a strided expand into 256B-stride rows for the pair gather.
"""

import os
import numpy as np
import ml_dtypes

N = 50000
E = 800000
FIN = 128
HID = 64
FOUT = 2
NCORE = 8
NPD = N // NCORE  # 6250 dst nodes per device

SBW = int(os.environ.get("GAT_SBW", 56))  # tiles per gather super-call
BAT = 7    # tiles per matmul batch
FT = 96    # tiles per scatter-psum flush round

ROW1_U16 = 128  # L1 table row: 65 bf16 logit | 4 bf16 yz | 1.0 | pad (256B)
SEL1 = 70       # u16 cols selected from an L1 pair half
VC1 = 65        # first value col (y0 y1 z0 z1 one)
ROW2_U16 = 8    # L2 compact row: 3 bf16 logit | 2 bf16 val | 1.0 | pad (16B)
SEL2 = 6
VC2 = 3
XR1_U16 = 66    # xr row: 65 bf16 | pad
XR2_U16 = 4     # xr2 row: 3 bf16 | pad
PAIRW = 256     # u16 width of a gathered pair (512B)


def _class_of(deg):
    return np.clip(np.ceil(deg / 4).astype(np.int64) * 4, 4, 124)


def build_schedule(edge_index):
    """Host-side scheduling. Returns everything the device program + pre/post
    processing needs. Pure numpy."""
    src = np.asarray(edge_index[0], dtype=np.int64)
    dst = np.asarray(edge_index[1], dtype=np.int64)
    loops = np.arange(N, dtype=np.int64)
    src = np.concatenate([src, loops])
    dst = np.concatenate([dst, loops])

    dev = dst // NPD
    ldst = dst - dev * NPD

    # per-device degree and class
    degs = np.zeros((NCORE, NPD), dtype=np.int64)
    for d in range(NCORE):
        degs[d] = np.bincount(ldst[dev == d], minlength=NPD)
    assert degs.max() <= 124, degs.max()
    K_nd = _class_of(degs)  # [NCORE, NPD]

    Ks = np.unique(K_nd)
    classes = []
    for K in Ks:
        K = int(K)
        m = 128 // K
        n_k = (K_nd == K).sum(axis=1)  # per device counts
        tiles = int(np.ceil(n_k.max() / m))
        if tiles == 0:
            continue
        classes.append(dict(K=K, m=m, tile_e=m * K, tiles=tiles))

    # slot layout: slot 0 reserved (zero row); then per class, tile-major
    ns = 1
    for c in classes:
        c["slot0"] = ns
        ns += c["tiles"] * c["m"]
    NS = ns

    # per-device: assign dsts to slots
    slot_of = np.full((NCORE, NPD), -1, dtype=np.int64)  # local dst -> slot
    node_at = np.full((NCORE, NS), -1, dtype=np.int64)   # slot -> local dst
    for d in range(NCORE):
        for c in classes:
            ids = np.where(K_nd[d] == c["K"])[0]
            s = c["slot0"] + np.arange(len(ids))
            slot_of[d, ids] = s
            node_at[d, s] = ids

    # gather index arrays: per super, dma_gather order n = k*128 + p
    NSP = -(-NS // 128) * 128
    gdev = src // NPD
    grow = gdev * NSP + slot_of[gdev, src - gdev * NPD]  # NSP-based table row

    supers = []      # metadata per super
    blocks = []      # (cls, t0, nch) in program order
    for c in classes:
        tiles = c["tiles"]
        t0 = 0
        while t0 < tiles:
            nch = min(SBW, tiles - t0)
            supers.append(dict(cls=c, t0=t0, nch=nch))
            blocks.append((c, t0, nch))
            t0 += nch

    # edge -> (class, tile, row-in-tile) mapping
    eslot = dev * NS + slot_of[dev, ldst]
    order = np.argsort(eslot, kind="stable")
    es = eslot[order]
    gr = grow[order]
    uniq, start_idx, counts = np.unique(es, return_index=True, return_counts=True)
    pos = np.arange(len(es)) - np.repeat(start_idx, counts)
    e_dev = (es // NS).astype(np.int64)
    e_slot = (es % NS).astype(np.int64)

    cls_of_slot = np.zeros(NS, dtype=np.int64)
    for ci, c in enumerate(classes):
        cls_of_slot[c["slot0"]: c["slot0"] + c["tiles"] * c["m"]] = ci
    cK = np.array([c["K"] for c in classes])
    cm = np.array([c["m"] for c in classes])
    cs0 = np.array([c["slot0"] for c in classes])
    ci = cls_of_slot[e_slot]
    rel = e_slot - cs0[ci]
    tt = rel // cm[ci]
    gg = rel % cm[ci]
    rr = gg * cK[ci] + pos
    assert (pos < cK[ci]).all()

    idx_parts = []
    mask_parts = []
    for c, t0, nch in blocks:
        cidx = classes.index(c)
        n_tot = 128 * nch
        valI = np.zeros((NCORE, n_tot), dtype=np.int64)
        valM = np.zeros((NCORE, n_tot), dtype=np.int64)
        msk = (ci == cidx) & (tt >= t0) & (tt < t0 + nch)
        n_e = (tt[msk] - t0) * 128 + rr[msk]
        rowv = gr[msk]
        dv = e_dev[msk]
        valI[dv, n_e] = rowv >> 1
        valM[dv, n_e] = rowv & 1
        # idx wrap int16: n -> [n%16, n//16], replicated over 8 core-groups
        w = valI.reshape(NCORE, n_tot // 16, 16).transpose(0, 2, 1)
        idx_parts.append(np.tile(w, (1, 8, 1)).astype(np.int16))
        # mask layout: position n -> [n%128, n//128], u32 0/0xFFFFFFFF
        mm = valM.reshape(NCORE, nch, 128).transpose(0, 2, 1)  # [C,128,nch]
        mask_parts.append((-mm).astype(np.int64).astype(np.uint32))
    idx1 = np.concatenate(idx_parts, axis=2)
    mask = np.concatenate(mask_parts, axis=2)

    # x rows in slot order (transposed), per device
    def slot_x(d, x):
        xs = np.zeros((NS, FIN), dtype=np.float32)
        valid = node_at[d] >= 0
        xs[valid] = x[d * NPD + node_at[d][valid]]
        return xs

    meta = dict(classes=classes, supers=supers, NS=NS, NSP=NSP, node_at=node_at,
                slot_of=slot_of, idx1=idx1, mask=mask, slot_x=slot_x)
    return meta


def prep_weights(Wl1, Wr1, att1, b1, Wl2, Wr2, att2, b2):
    """Host weight transforms. Returns dict of constant arrays."""
    perm1 = np.argsort(att1 < 0, kind="stable")
    a1p = np.abs(att1[perm1])
    npos1 = int((att1 >= 0).sum())
    wtab_att = np.zeros((FIN, 65), dtype=np.float32)
    wtab_att[:, :64] = Wl1[:, perm1] * a1p[None, :]
    wtab_att[:, 64] = 0.25 * (Wl1 @ att1)
    w_yz = np.concatenate([Wl1 @ Wl2, Wl1 @ Wr2], axis=1).astype(np.float32)  # [128,4]
    wxr = np.zeros((FIN, 65), dtype=np.float32)
    wxr[:, :64] = Wr1[:, perm1] * a1p[None, :]
    wxr[:, 64] = 0.25 * (Wr1 @ att1)

    cy = b1 @ Wl2  # [2]
    cz = b1 @ Wr2
    perm2 = np.argsort(att2 < 0, kind="stable")
    a2p = np.abs(att2[perm2])
    npos2 = int((att2 >= 0).sum())
    return dict(
        wtab_att=wtab_att, w_yz=w_yz, wxr=wxr, npos1=npos1,
        cy=cy, cz=cz, perm2=perm2, a2p=a2p, npos2=npos2,
        att2=att2, b2=b2,
    )


def make_oh_consts(classes):
    """Constant block-one-hot matrices per class (host-built)."""
    ohst = []  # stacked lhsT per class: [m x tile_e] -> [128, tile_e]
    ohb = []   # OH_exd replicated BAT times: [tile_e, BAT*m] -> [128, BAT*m]
    for c in classes:
        K, m, te = c["K"], c["m"], c["tile_e"]
        st = np.zeros((128, te), dtype=np.float32)
        for g in range(m):
            st[g, g * K:(g + 1) * K] = 1.0  # OH_dxe rows
        ohst.append(st)
        ex = np.zeros((te, m), dtype=np.float32)
        for g in range(m):
            ex[g * K:(g + 1) * K, g] = 1.0
        blk = np.zeros((128, BAT * m), dtype=np.float32)
        for j in range(BAT):
            blk[:te, j * m:(j + 1) * m] = ex
        ohb.append(blk)
    return ohst, ohb


# --------------------------------------------------------------------------
# numpy mock of the exact device pipeline (bf16 rounding included)
# --------------------------------------------------------------------------

def mock_run(meta, W, x):
    bf = ml_dtypes.bfloat16
    NS, NSP = meta["NS"], meta["NSP"]
    classes = meta["classes"]

    def bfr(a):
        return a.astype(bf).astype(np.float32)

    # layer-1 table: [NCORE*NSP, 70] (65 logit | 4 yz | one), bf16-rounded
    tables = []
    xr_rows_all = []
    for d in range(NCORE):
        xs = bfr(meta["slot_x"](d, x))
        tab = np.zeros((NSP, 70), dtype=np.float32)
        tab[:NS, 0:65] = bfr(xs @ bfr(W["wtab_att"]))
        tab[:NS, 65:69] = bfr(xs @ bfr(W["w_yz"]))
        tab[:NS, 69] = 1.0
        tab[0] = 0.0
        tables.append(tab)
        xr_r = np.zeros((NSP, 65), np.float32)
        xr_r[:NS] = bfr(xs @ bfr(W["wxr"]))
        xr_rows_all.append(xr_r)
    table_all = np.concatenate(tables, axis=0)

    def unwrap_idx(idx16, n_tot, off16):
        w = idx16[:, off16:off16 + n_tot // 16]
        return w[:16].T.reshape(-1).astype(np.int64)

    def unwrap_mask(mk, n_tot, offm):
        m = mk[:, offm:offm + n_tot // 128]
        return (m.T.reshape(-1).astype(np.int64) & 1)

    def edge_pass(tabL, xrL, NP_, NF, NV):
        # tabL: [NCORE*NSP, NLOG+NV]; logit cols 0:NF+1, value cols NF+1:
        NLOG = NF + 1
        aggs = [np.zeros((NSP, NV), np.float32) for _ in range(NCORE)]
        for d in range(NCORE):
            off16 = 0
            offm = 0
            for c in classes:
                K, m, te, tiles = c["K"], c["m"], c["tile_e"], c["tiles"]
                t0 = 0
                while t0 < tiles:
                    nch = min(SBW, tiles - t0)
                    n_tot = 128 * nch
                    iP = unwrap_idx(meta["idx1"][d], n_tot, off16)
                    mP = unwrap_mask(meta["mask"][d], n_tot, offm)
                    rows = tabL[2 * iP + mP].reshape(nch, 128, -1)
                    g_of = np.arange(te) // K
                    xr_t = np.stack([xrL[d][c["slot0"] + (t0 + ch) * m:
                                            c["slot0"] + (t0 + ch + 1) * m]
                                     for ch in range(nch)], axis=0)
                    H = rows[:, :te, 0:NLOG] + xr_t[:, g_of, 0:NLOG]
                    relu = bfr(np.maximum(H[:, :, 0:NF], 0.0))
                    dpos = relu[:, :, :NP_].sum(2) - relu[:, :, NP_:].sum(2)
                    sv = bfr(np.exp(0.8 * (dpos + H[:, :, NF])))
                    vals = rows[:, :te, NLOG:NLOG + NV]
                    contrib = sv[:, :, None] * vals
                    for ch in range(nch):
                        sl = c["slot0"] + (t0 + ch) * m
                        for g in range(m):
                            aggs[d][sl + g] += contrib[ch, g * K:(g + 1) * K].sum(0)
                    off16 += n_tot // 16
                    offm += nch
                    t0 += nch
        return aggs

    aggs = edge_pass(table_all, xr_rows_all, W["npos1"], 64, 5)
    l2_tabs, xr2_all = [], []
    for d in range(NCORE):
        rcp = 1.0 / (aggs[d][:, 4] + 1e-30)
        sc = aggs[d][:, :4] * rcp[:, None]
        xl2 = sc[:, 0:2] + W["cy"][None, :]
        xr2v = sc[:, 2:4] + W["cz"][None, :]
        p2 = 0.25 * (xl2 @ W["att2"])
        q2 = 0.25 * (xr2v @ W["att2"])
        l2 = np.zeros((NSP, 6), np.float32)
        l2[:, 0:2] = bfr(xl2[:, W["perm2"]] * W["a2p"][None, :])
        l2[:, 2] = bfr(p2)
        l2[:, 3:5] = bfr(xl2)
        l2[:, 5] = 1.0
        l2[0] = 0.0
        l2_tabs.append(l2)
        xr2 = np.zeros((NSP, 3), np.float32)
        xr2[:, 0:2] = bfr(xr2v[:, W["perm2"]] * W["a2p"][None, :])
        xr2[:, 2] = bfr(q2)
        xr2_all.append(xr2)
    l2_all = np.concatenate(l2_tabs, axis=0)

    aggs2 = edge_pass(l2_all, xr2_all, W["npos2"], 2, 3)
    out = np.zeros((N, FOUT), dtype=np.float32)
    for d in range(NCORE):
        rcp = 1.0 / (aggs2[d][:, 2] + 1e-30)
        o2 = aggs2[d][:, :2] * rcp[:, None] + W["b2"][None, :]
        valid = meta["node_at"][d] >= 0
        out[d * NPD + meta["node_at"][d][valid]] = o2[:NS][valid]
    return out


# --------------------------------------------------------------------------
# device program (Bass/Tile)
# --------------------------------------------------------------------------

import concourse.bass as bass
import concourse.bacc as bacc_mod
import concourse.mybir as mybir
import concourse.tile as tile
from concourse.bass_utils import run_bass_kernel_spmd


F32 = mybir.dt.float32
BF16 = mybir.dt.bfloat16
U16 = mybir.dt.uint16
U32 = mybir.dt.uint32
I16 = mybir.dt.int16
AF = mybir.ActivationFunctionType
ALU = mybir.AluOpType
AX = mybir.AxisListType


def _ceil(a, b):
    return -(-a // b)


def build_program(meta, W):
    classes = meta["classes"]
    NS = meta["NS"]
    NSP = _ceil(NS, 128) * 128          # padded slot count
    HPAIR = NCORE * NSP // 2            # pair rows in the gathered tables
    npos1 = W["npos1"]
    nneg1 = 64 - npos1
    npos2 = W["npos2"]
    nneg2 = 2 - npos2
    perm2 = W["perm2"]
    a2p = W["a2p"]
    cy, cz = W["cy"], W["cz"]
    att2, b2 = W["att2"], W["b2"]

    ohst_np, ohb_np = make_oh_consts(classes)
    OHST_W = sum(a.shape[1] for a in ohst_np)
    OHB_W = sum(a.shape[1] for a in ohb_np)

    TOT = NCORE * NSP
    nc = bacc_mod.Bacc(None)
    xT = nc.declare_dram_parameter("xT", [FIN, TOT], BF16, isOutput=False)
    xTown = nc.declare_dram_parameter("xTown", [FIN, NSP], BF16, isOutput=False)
    wtab = nc.declare_dram_parameter("wtab", [FIN, 65], BF16, isOutput=False)
    wyz = nc.declare_dram_parameter("wyz", [FIN, 4], BF16, isOutput=False)
    wxr = nc.declare_dram_parameter("wxr", [FIN, 65], BF16, isOutput=False)
    IDXW = meta["idx1"].shape[2]
    MSKW = meta["mask"].shape[2]
    idx1p = nc.declare_dram_parameter("idx1", [128, IDXW], I16, isOutput=False)
    maskp = nc.declare_dram_parameter("mask", [128, MSKW], U32, isOutput=False)
    eyep = nc.declare_dram_parameter("eye", [128, 128], BF16, isOutput=False)
    ohstp = nc.declare_dram_parameter("ohst", [128, OHST_W], BF16, isOutput=False)
    ohbp = nc.declare_dram_parameter("ohb", [128, OHB_W], BF16, isOutput=False)
    out2 = nc.declare_dram_parameter("out2", [NSP, 2], F32, isOutput=True)

    with tile.TileContext(nc) as tc:
        with (
            tc.tile_pool(name="dram", bufs=1, space="DRAM") as dram,
            tc.tile_pool(name="cpool", bufs=1) as cpool,
            tc.tile_pool(name="sb", bufs=int(os.environ.get("GAT_SBB", 3))) as sb,
            tc.tile_pool(name="sb2", bufs=int(os.environ.get("GAT_SB2", 3))) as sb2,
            tc.tile_pool(name="ps", bufs=int(os.environ.get("GAT_PSB", 5)), space="PSUM") as ps,
            tc.tile_pool(name="psS", bufs=int(os.environ.get("GAT_PSS", 3)), space="PSUM") as psSp,
        ):
            # AG2 split row: end of the first class whose cumulative rows
            # reach ~80% of NS (chunk A all-gathers + expands early, under
            # the tail of edge pass 1)
            rA = NSP
            splitc = len(classes) - 1
            for i, c in enumerate(classes):
                end = c["slot0"] + c["tiles"] * c["m"]
                if end >= 0.8 * NS:
                    rA, splitc = end, i
                    break

            table = dram.tile([HPAIR, PAIRW], U16)
            tabN = table.rearrange("q (t w) -> (q t) w", t=2)
            l2slice = dram.tile([NSP, ROW2_U16], U16)
            l2compA = dram.tile([NCORE * rA, ROW2_U16], U16, addr_space="Shared")
            l2compB = dram.tile([NCORE * (NSP - rA), ROW2_U16], U16,
                                addr_space="Shared")
            l2fat = dram.tile([HPAIR, PAIRW], U16)
            l2fatD = l2fat.rearrange("(d r) (t w) -> d (r t) w", d=NCORE, t=2)
            xr_dram = dram.tile([NSP, XR1_U16], U16)
            xr2_dram = dram.tile([NSP, XR2_U16], U16)

            # ---------------- consts ----------------
            wtab_sb = cpool.tile([FIN, 65], BF16)
            nc.sync.dma_start(wtab_sb[:, :], wtab[:, :])
            wyz_sb = cpool.tile([FIN, 4], BF16)
            nc.sync.dma_start(wyz_sb[:, :], wyz[:, :])
            wxr_sb = cpool.tile([FIN, 65], BF16)
            nc.sync.dma_start(wxr_sb[:, :], wxr[:, :])
            ohst_sb = cpool.tile([128, OHST_W], BF16)
            nc.sync.dma_start(ohst_sb[:, :], ohstp[:, :])
            ohb_sb = cpool.tile([128, OHB_W], BF16)
            nc.sync.dma_start(ohb_sb[:, :], ohbp[:, :])
            eye_sb = cpool.tile([128, 128], BF16)
            nc.sync.dma_start(eye_sb[:, :], eyep[:, :])
            idx_sb = cpool.tile([128, IDXW], I16)
            nc.sync.dma_start(idx_sb[:, :], idx1p[:, :])
            msk_sb = cpool.tile([128, MSKW], U32)
            nc.sync.dma_start(msk_sb[:, :], maskp[:, :])

            # ------- dense phase: full table computed locally (replicated),
            # plus own-slice xr rows -------
            DG = 6       # chunks per psum round
            GW = 12      # chunks per load/store macro-group
            assert (TOT // 128) % GW == 0 and (NSP // 128) % 3 == 0
            for c0 in range(0, TOT // 128, GW):
                xch = sb.tile([128, GW * 128], BF16, tag="xch")
                nc.sync.dma_start(xch[:, :], xT[:, c0 * 128:(c0 + GW) * 128])
                rows = sb.tile([128, GW * ROW1_U16], U16, tag="rows")
                rv = rows.bitcast(BF16).rearrange("p (g w) -> p g w", w=ROW1_U16)
                for r in range(GW // DG):
                    ps_d = ps.tile([128, BAT * 65], F32, tag="psH")
                    pdv = ps_d[:, 0:DG * 69].rearrange("p (g w) -> p g w", w=69)
                    for g in range(DG):
                        gg = r * DG + g
                        lh = xch[:, gg * 128:(gg + 1) * 128]
                        nc.tensor.matmul(out=pdv[:, g, 0:65], lhsT=lh,
                                         rhs=wtab_sb[:, :], start=True, stop=True)
                        nc.tensor.matmul(out=pdv[:, g, 65:69], lhsT=lh,
                                         rhs=wyz_sb[:, :], start=True, stop=True)
                    nc.scalar.activation(rv[:, r * DG:(r + 1) * DG, 0:69],
                                         pdv[:, :, 0:69], AF.Copy)
                nc.vector.memset(rv[:, :, 69:70], 1.0)
                nc.sync.dma_start(
                    tabN[c0 * 128:(c0 + GW) * 128, 0:70]
                    .rearrange("(g p) w -> p g w", p=128),
                    rows[:, :].rearrange("p (g w) -> p g w", w=ROW1_U16)[:, :, 0:70])
            for c0 in range(0, NSP // 128, 3):
                xch = sb.tile([128, 3 * 128], BF16, tag="xcho")
                nc.scalar.dma_start(xch[:, :], xTown[:, c0 * 128:(c0 + 3) * 128])
                ps_x = ps.tile([128, BAT * 65], F32, tag="psH")
                pxv = ps_x[:, 0:3 * 65].rearrange("p (g w) -> p g w", w=65)
                for g in range(3):
                    nc.tensor.matmul(out=pxv[:, g, :],
                                     lhsT=xch[:, g * 128:(g + 1) * 128],
                                     rhs=wxr_sb[:, :], start=True, stop=True)
                xrr = sb.tile([128, 3 * XR1_U16], U16, tag="xrr")
                xv = xrr.bitcast(BF16).rearrange("p (g w) -> p g w", w=XR1_U16)
                nc.scalar.activation(xv[:, :, 0:65], pxv[:, :, :], AF.Copy)
                nc.vector.memset(xv[:, :, 65:66], 0.0)
                nc.scalar.dma_start(
                    xr_dram[c0 * 128:(c0 + 3) * 128, :]
                    .rearrange("(g p) w -> p g w", p=128),
                    xrr.rearrange("p (g w) -> p g w", w=XR1_U16))

            # zero row 0 of the table (the reserved all-zero row)
            zr = sb.tile([1, ROW1_U16], U16, tag="zr")
            nc.vector.memset(zr[:, :], 0)
            nc.sync.dma_start(tabN[0:1, :], zr[:, :])

            # zero row 0 of l2slice before edge pass 1 starts writing it
            zr2 = sb.tile([1, ROW2_U16], U16, tag="zr2")
            nc.vector.memset(zr2[:, :], 0)
            nc.sync.dma_start(l2slice[0:1, :], zr2[:, :])

            def ag2_chunk(which):
                if which == 0:
                    r0, r1, comp = 0, rA, l2compA
                else:
                    r0, r1, comp = rA, NSP, l2compB
                nc.gpsimd.collective_compute(
                    "AllGather", ALU.bypass,
                    replica_groups=[list(range(NCORE))],
                    ins=[l2slice[r0:r1, :]], outs=[comp[:, :]],
                )
                nc.sync.dma_start(
                    l2fatD[:, r0:r1, 0:ROW2_U16],
                    comp.rearrange("(d r) w -> d r w", d=NCORE))

            # ---------------- edge pass helper ----------------
            def edge_pass(lay):
                if lay == 1:
                    tabT, xrT = table, xr_dram
                    SW, VC, XRW = SEL1, VC1, XR1_U16
                    NP_, NN_, NF = npos1, nneg1, 64
                    NV = 5
                else:
                    tabT, xrT = l2fat, xr2_dram
                    SW, VC, XRW = SEL2, VC2, XR2_U16
                    NP_, NN_, NF = npos2, nneg2, 2
                    NV = 3
                NLOG = NF + 1  # logit cols incl p/q col
                idx_off = 0
                msk_off = 0
                ohst_off = 0
                ohb_off = 0
                for cls_i, cls in enumerate(classes):
                    K, m, te, tiles, slot0 = (cls["K"], cls["m"], cls["tile_e"],
                                              cls["tiles"], cls["slot0"])
                    psS = None
                    f_t0 = 0

                    def flush(ntl):
                        # flush tiles [f_t0, f_t0+ntl) of this class
                        P = psS[:, 0:FT * NV].rearrange("p (j v) -> p j v", v=NV)
                        rcpi = sb.tile([128, FT], F32, tag="rcpi")
                        nc.vector.tensor_scalar(
                            out=rcpi[0:m, 0:ntl], in0=P[0:m, 0:ntl, NV - 1],
                            scalar1=1e-30, scalar2=None, op0=ALU.add)
                        rcp = sb.tile([128, FT], F32, tag="rcp")
                        nc.vector.reciprocal(rcp[0:m, 0:ntl], rcpi[0:m, 0:ntl])
                        sc = sb.tile([128, FT * 4], F32, tag="sc")
                        scv = sc.rearrange("p (j v) -> p j v", v=4)
                        nc.vector.tensor_tensor(
                            out=scv[0:m, 0:ntl, 0:NV - 1], in0=P[0:m, 0:ntl, 0:NV - 1],
                            in1=rcp[0:m, 0:ntl].rearrange("p (j o) -> p j o", o=1)
                                .to_broadcast([m, ntl, NV - 1]),
                            op=ALU.mult)
                        r0 = slot0 + f_t0 * m
                        nrows = ntl * m
                        if lay == 1:
                            l2r = sb.tile([128, FT * ROW2_U16], U16, tag="l2r")
                            lb = l2r.bitcast(BF16).rearrange(
                                "p (j v) -> p j v", v=ROW2_U16)
                            x2r = sb.tile([128, FT * XR2_U16], U16, tag="x2r")
                            xb = x2r.bitcast(BF16).rearrange(
                                "p (j v) -> p j v", v=XR2_U16)
                            t1 = sb.tile([128, FT], F32, tag="t1")
                            t2 = sb.tile([128, FT], F32, tag="t2")
                            for cc in range(2):
                                nc.vector.tensor_scalar(
                                    out=lb[0:m, 0:ntl, cc], in0=scv[0:m, 0:ntl, perm2[cc]],
                                    scalar1=float(a2p[cc]),
                                    scalar2=float(a2p[cc] * cy[perm2[cc]]),
                                    op0=ALU.mult, op1=ALU.add)
                                nc.vector.tensor_scalar(
                                    out=xb[0:m, 0:ntl, cc], in0=scv[0:m, 0:ntl, 2 + perm2[cc]],
                                    scalar1=float(a2p[cc]),
                                    scalar2=float(a2p[cc] * cz[perm2[cc]]),
                                    op0=ALU.mult, op1=ALU.add)
                            nc.vector.tensor_scalar(
                                out=t1[0:m, 0:ntl], in0=scv[0:m, 0:ntl, 0],
                                scalar1=float(0.25 * att2[0]),
                                scalar2=float(0.25 * (att2 @ cy)),
                                op0=ALU.mult, op1=ALU.add)
                            nc.vector.tensor_scalar(
                                out=t2[0:m, 0:ntl], in0=scv[0:m, 0:ntl, 1],
                                scalar1=float(0.25 * att2[1]), scalar2=None, op0=ALU.mult)
                            nc.vector.tensor_tensor(
                                out=lb[0:m, 0:ntl, 2], in0=t1[0:m, 0:ntl],
                                in1=t2[0:m, 0:ntl], op=ALU.add)
                            nc.vector.tensor_scalar(
                                out=t1[0:m, 0:ntl], in0=scv[0:m, 0:ntl, 2],
                                scalar1=float(0.25 * att2[0]),
                                scalar2=float(0.25 * (att2 @ cz)),
                                op0=ALU.mult, op1=ALU.add)
                            nc.vector.tensor_scalar(
                                out=t2[0:m, 0:ntl], in0=scv[0:m, 0:ntl, 3],
                                scalar1=float(0.25 * att2[1]), scalar2=None, op0=ALU.mult)
                            nc.vector.tensor_tensor(
                                out=xb[0:m, 0:ntl, 2], in0=t1[0:m, 0:ntl],
                                in1=t2[0:m, 0:ntl], op=ALU.add)
                            nc.vector.memset(xb[0:m, 0:ntl, 3], 0.0)
                            for cc in range(2):
                                nc.vector.tensor_scalar(
                                    out=lb[0:m, 0:ntl, 3 + cc],
                                    in0=scv[0:m, 0:ntl, cc],
                                    scalar1=float(cy[cc]), scalar2=None, op0=ALU.add)
                            nc.vector.memset(lb[0:m, 0:ntl, 5], 1.0)
                            for (buf, dstt, w) in ((l2r, l2slice, ROW2_U16),
                                                   (x2r, xr2_dram, XR2_U16)):
                                dst_ap = dstt[r0:r0 + nrows, 0:w] \
                                    .rearrange("(j p) w -> p j w", p=m)
                                src_ap = buf.rearrange("p (j v) -> p j v", v=w)[
                                    0:m, 0:ntl, :]
                                nc.sync.dma_start(dst_ap, src_ap)
                        else:
                            o2 = sb.tile([128, FT * 2], F32, tag="o2")
                            o2v = o2.rearrange("p (j v) -> p j v", v=2)
                            for cc in range(2):
                                nc.vector.tensor_scalar(
                                    out=o2v[0:m, 0:ntl, cc], in0=scv[0:m, 0:ntl, cc],
                                    scalar1=float(b2[cc]), scalar2=None, op0=ALU.add)
                            dst_ap = out2[r0:r0 + nrows, :] \
                                .rearrange("(j p) w -> p j w", p=m)
                            nc.sync.dma_start(dst_ap, o2v[0:m, 0:ntl, :])

                    t0 = 0
                    while t0 < tiles:
                        nch = min(SBW, tiles - t0)
                        STP = sb2.tile([128, SBW * PAIRW], U16, tag="STP")
                        c16 = idx_off // 16
                        nc.gpsimd.dma_gather(
                            out_ap=STP[:, 0:nch * PAIRW]
                            .rearrange("p (k w) -> p k w", w=PAIRW),
                            in_ap=tabT[0:HPAIR, :],
                            idxs_ap=idx_sb[:, c16:c16 + 8 * nch],
                            num_idxs=128 * nch, num_idxs_reg=128 * nch,
                            elem_size=PAIRW, single_packet=False)
                        # pair select: STS = lo ^ ((lo ^ hi) & mask), u32 ALU
                        SW2 = SW // 2
                        STP32 = STP.bitcast(U32).rearrange(
                            "p (k w) -> p k w", w=PAIRW // 2)
                        STS = sb2.tile([128, SBW * SW], U16, tag="STS")
                        sv_ = STS.bitcast(U32).rearrange("p (k w) -> p k w", w=SW2)
                        nc.vector.tensor_tensor(
                            out=sv_[:, 0:nch, :], in0=STP32[:, 0:nch, 0:SW2],
                            in1=STP32[:, 0:nch, 64:64 + SW2], op=ALU.bitwise_xor)
                        nc.vector.tensor_tensor(
                            out=sv_[:, 0:nch, :], in0=sv_[:, 0:nch, :],
                            in1=msk_sb[:, msk_off:msk_off + nch]
                            .rearrange("p (k o) -> p k o", o=1)
                            .to_broadcast([128, nch, SW2]),
                            op=ALU.bitwise_and)
                        nc.vector.tensor_tensor(
                            out=sv_[:, 0:nch, :], in0=sv_[:, 0:nch, :],
                            in1=STP32[:, 0:nch, 0:SW2], op=ALU.bitwise_xor)
                        STSb = STS.bitcast(BF16).rearrange("p (k w) -> p k w", w=SW)
                        # xr rows for these tiles
                        xrst = sb2.tile([128, SBW * XR1_U16], U16, tag="xrst")
                        xru = xrst.rearrange("p (k w) -> p k w", w=XR1_U16)
                        r0 = slot0 + t0 * m
                        nc.scalar.dma_start(
                            xru[0:m, 0:nch, 0:XRW],
                            xrT[r0:r0 + nch * m, 0:XRW]
                            .rearrange("(c g) w -> g c w", g=m))
                        xrb = xrst.bitcast(BF16).rearrange("p (k w) -> p k w", w=XR1_U16)
                        for b in range(_ceil(nch, BAT)):
                            nb = min(BAT, nch - b * BAT)
                            bs = slice(b * BAT, b * BAT + nb)
                            psH = ps.tile([128, BAT * NLOG], F32, tag="psH")
                            pHv = psH.rearrange("p (b w) -> p b w", w=NLOG)
                            nc.tensor.matmul(
                                out=pHv[0:te, 0:nb, :],
                                lhsT=eye_sb[0:te, 0:te],
                                rhs=STSb[0:te, bs, 0:NLOG],
                                start=True, stop=False)
                            nc.tensor.matmul(
                                out=pHv[0:te, 0:nb, :],
                                lhsT=ohst_sb[0:m, ohst_off:ohst_off + te],
                                rhs=xrb[0:m, bs, 0:NLOG],
                                start=False, stop=True)
                            Hr = sb.tile([128, BAT * NF], BF16, tag=f"Hr{lay}")
                            Hv = Hr.rearrange("p (b w) -> p b w", w=NF)
                            nc.scalar.activation(
                                Hv[0:te, 0:nb, :], pHv[0:te, 0:nb, 0:NF], AF.Relu)
                            dt = sb.tile([128, BAT], F32, tag=f"dt{lay}")
                            if NP_ > 0 and NN_ > 0:
                                Ap = sb.tile([128, BAT], F32, tag=f"Ap{lay}")
                                An = sb.tile([128, BAT], F32, tag=f"An{lay}")
                                nc.vector.tensor_reduce(
                                    out=Ap[0:te, 0:nb], in_=Hv[0:te, 0:nb, 0:NP_],
                                    axis=AX.X, op=ALU.add)
                                nc.vector.tensor_reduce(
                                    out=An[0:te, 0:nb], in_=Hv[0:te, 0:nb, NP_:NF],
                                    axis=AX.X, op=ALU.add)
                                nc.vector.tensor_tensor(
                                    out=dt[0:te, 0:nb], in0=Ap[0:te, 0:nb],
                                    in1=An[0:te, 0:nb], op=ALU.subtract)
                            else:
                                nc.vector.tensor_reduce(
                                    out=dt[0:te, 0:nb], in_=Hv[0:te, 0:nb, 0:NF],
                                    axis=AX.X, op=ALU.add)
                                if NN_ > 0:
                                    nc.vector.tensor_scalar(
                                        out=dt[0:te, 0:nb], in0=dt[0:te, 0:nb],
                                        scalar1=-1.0, scalar2=None, op0=ALU.mult)
                            ep = sb.tile([128, BAT], F32, tag=f"ep{lay}")
                            nc.vector.tensor_tensor(
                                out=ep[0:te, 0:nb], in0=dt[0:te, 0:nb],
                                in1=pHv[0:te, 0:nb, NF], op=ALU.add)
                            sB = sb.tile([128, BAT], F32, tag=f"sB{lay}")
                            nc.scalar.activation(
                                sB[0:te, 0:nb], ep[0:te, 0:nb], AF.Exp, scale=0.8)
                            soh = sb.tile([128, BAT * 33], BF16, tag=f"soh{lay}")
                            sohv = soh.rearrange("p (b w) -> p b w", w=33)
                            nc.vector.tensor_tensor(
                                out=sohv[0:te, 0:nb, 0:m],
                                in0=ohb_sb[0:te, ohb_off:ohb_off + nb * m]
                                .rearrange("p (b w) -> p b w", w=m),
                                in1=sB[0:te, 0:nb]
                                .rearrange("p (b o) -> p b o", o=1)
                                .to_broadcast([te, nb, m]),
                                op=ALU.mult)
                            for j in range(nb):
                                tg = t0 + b * BAT + j
                                jj = tg - f_t0
                                if jj == 0:
                                    psS = psSp.tile([128, FT * NV], F32,
                                                    tag="psS")
                                kabs = b * BAT + j
                                nc.tensor.matmul(
                                    out=psS[0:m, jj * NV:(jj + 1) * NV],
                                    lhsT=sohv[0:te, j, 0:m],
                                    rhs=STSb[0:te, kabs, VC:VC + NV],
                                    start=True, stop=True)
                                if jj == FT - 1 or tg == tiles - 1:
                                    flush(jj + 1)
                                    f_t0 = tg + 1
                                    psS = None
                        idx_off += 128 * nch
                        msk_off += nch
                        t0 += nch
                    ohst_off += te
                    ohb_off += BAT * m
                    if lay == 1 and cls_i == splitc:
                        ag2_chunk(0)

            edge_pass(1)
            ag2_chunk(1)
            edge_pass(2)

    return nc, NSP


def run_device(meta, W, x, trace=False):
    nc, NSP = build_program(meta, W)
    NS = meta["NS"]
    assert NSP == meta["NSP"]
    classes = meta["classes"]
    ohst_np, ohb_np = make_oh_consts(classes)
    ohst = np.concatenate(ohst_np, axis=1).astype(ml_dtypes.bfloat16)
    ohb = np.concatenate(ohb_np, axis=1).astype(ml_dtypes.bfloat16)

    bf = ml_dtypes.bfloat16
    slices = []
    for d in range(NCORE):
        xsp = np.zeros((NSP, FIN), dtype=np.float32)
        xsp[:NS] = meta["slot_x"](d, x)
        slices.append(xsp)
    xall_T = np.ascontiguousarray(
        np.concatenate(slices, axis=0).T).astype(bf)  # [FIN, NCORE*NSP]

    in_maps = []
    for d in range(NCORE):
        im = dict(
            xT=xall_T,
            xTown=np.ascontiguousarray(slices[d].T).astype(bf),
            wtab=W["wtab_att"].astype(bf),
            wyz=W["w_yz"].astype(bf),
            wxr=W["wxr"].astype(bf),
            idx1=meta["idx1"][d],
            mask=meta["mask"][d],
            eye=np.eye(128, dtype=bf),
            ohst=ohst,
            ohb=ohb,
        )
        in_maps.append(im)

    if not nc.is_finalized():
        nc.finalize()
    res = run_bass_kernel_spmd(nc, in_maps, list(range(NCORE)), trace=trace)
    outs = res.results
    out = np.zeros((N, FOUT), dtype=np.float32)
    for d in range(NCORE):
        o = outs[d]["out2"]
        valid = meta["node_at"][d] >= 0
        out[d * NPD + meta["node_at"][d][valid]] = o[:NS][valid]
    return out, res


# --------------------------------------------------------------------------
# entry
# --------------------------------------------------------------------------

def kernel(**inputs):
    x = np.asarray(inputs["x"], dtype=np.float32)
    meta = build_schedule(np.asarray(inputs["edge_index"]))
    W = prep_weights(
        np.asarray(inputs["Wl1"], np.float32), np.asarray(inputs["Wr1"], np.float32),
        np.asarray(inputs["att1"], np.float32), np.asarray(inputs["b1"], np.float32),
        np.asarray(inputs["Wl2"], np.float32), np.asarray(inputs["Wr2"], np.float32),
        np.asarray(inputs["att2"], np.float32), np.asarray(inputs["b2"], np.float32),
    )
    if os.environ.get("GAT_MOCK"):
        return mock_run(meta, W, x)
    out, _res = run_device(meta, W, x)
    return out


if __name__ == "__main__":
    pass


# revision 29
# speedup vs baseline: 1.0004x; 1.0004x over previous
"""GATv2 (2-layer, heads=1) on 8 trn2 NeuronCores.

Strategy (dst-sharded, degree-class scheduled, pair-packed gathers):
  - Nodes partitioned across 8 cores by dst id (6250 each). Edges (with self
    loops) grouped by dst; each dst padded to a degree-class K (multiple of 4).
    Per class: tiles of m=floor(128/K) dsts x K slots = tile_e edge slots.
  - Layer-1 logit:  e = att.leaky_relu(xl[src]+xr[dst])
                    = 0.2*(p_j+q_i) + 0.8*(relu-sum over sign-sorted, |att|-
                      scaled feature columns), with p=att.xl, q=att.xr.
    Softmax max-subtraction is skipped (shift-invariant; exponents are small).
  - h (layer-1 output) is never materialized: layer 2 only needs
    xl2 = h@Wl2, xr2 = h@Wr2, which are linear in the layer-1 aggregates of
    y = xl@Wl2, z = xl@Wr2 (2 cols each).
  - The layer-1 node table (256B/row: 65 bf16 logit cols | 4 bf16 y,z |
    1.0) is computed REPLICATED on every core from a host-shipped full
    bf16 x in slot order -- no layer-1 collective at all.
  - Per-edge gather: ONE 512B descriptor per edge slot fetches a PAIR of
    256B table rows (pair idx = row>>1 fits int16); the wanted half is
    selected on-chip with 3 DVE u32 bitwise ops and a host parity mask.
  - Per-dst aggregation via small per-tile matmuls with a constant block
    one-hot scaled by s=exp(e), all bf16 into f32 PSUM.
  - Layer-2 node table is compact bf16 (16B/row): two chunked AllGathers
    (0.8MB total, the first overlapped under the tail of edge pass 1),
    then a strided expand into 256B-stride rows for the pair gather.
"""

import os
import numpy as np
import ml_dtypes

N = 50000
E = 800000
FIN = 128
HID = 64
FOUT = 2
NCORE = 8
NPD = N // NCORE  # 6250 dst nodes per device

SBW = int(os.environ.get("GAT_SBW", 56))  # tiles per gather super-call
BAT = 7    # tiles per matmul batch
FT = 96    # tiles per scatter-psum flush round

ROW1_U16 = 128  # L1 table row: 65 bf16 logit | 4 bf16 yz | 1.0 | pad (256B)
SEL1 = 70       # u16 cols selected from an L1 pair half
VC1 = 65        # first value col (y0 y1 z0 z1 one)
ROW2_U16 = 8    # L2 compact row: 3 bf16 logit | 2 bf16 val | 1.0 | pad (16B)
SEL2 = 6
VC2 = 3
XR1_U16 = 66    # xr row: 65 bf16 | pad
XR2_U16 = 4     # xr2 row: 3 bf16 | pad
PAIRW = 256     # u16 width of a gathered pair (512B)


def _class_of(deg):
    return np.clip(np.ceil(deg / 4).astype(np.int64) * 4, 4, 124)


def build_schedule(edge_index):
    """Host-side scheduling. Returns everything the device program + pre/post
    processing needs. Pure numpy."""
    src = np.asarray(edge_index[0], dtype=np.int64)
    dst = np.asarray(edge_index[1], dtype=np.int64)
    loops = np.arange(N, dtype=np.int64)
    src = np.concatenate([src, loops])
    dst = np.concatenate([dst, loops])

    dev = dst // NPD
    ldst = dst - dev * NPD

    # per-device degree and class
    degs = np.zeros((NCORE, NPD), dtype=np.int64)
    for d in range(NCORE):
        degs[d] = np.bincount(ldst[dev == d], minlength=NPD)
    assert degs.max() <= 124, degs.max()
    K_nd = _class_of(degs)  # [NCORE, NPD]

    Ks = np.unique(K_nd)
    classes = []
    for K in Ks:
        K = int(K)
        m = 128 // K
        n_k = (K_nd == K).sum(axis=1)  # per device counts
        tiles = int(np.ceil(n_k.max() / m))
        if tiles == 0:
            continue
        classes.append(dict(K=K, m=m, tile_e=m * K, tiles=tiles))

    # slot layout: slot 0 reserved (zero row); then per class, tile-major
    ns = 1
    for c in classes:
        c["slot0"] = ns
        ns += c["tiles"] * c["m"]
    NS = ns

    # per-device: assign dsts to slots
    slot_of = np.full((NCORE, NPD), -1, dtype=np.int64)  # local dst -> slot
    node_at = np.full((NCORE, NS), -1, dtype=np.int64)   # slot -> local dst
    for d in range(NCORE):
        for c in classes:
            ids = np.where(K_nd[d] == c["K"])[0]
            s = c["slot0"] + np.arange(len(ids))
            slot_of[d, ids] = s
            node_at[d, s] = ids

    # gather index arrays: per super, dma_gather order n = k*128 + p
    NSP = -(-NS // 128) * 128
    gdev = src // NPD
    grow = gdev * NSP + slot_of[gdev, src - gdev * NPD]  # NSP-based table row

    supers = []      # metadata per super
    blocks = []      # (cls, t0, nch) in program order
    for c in classes:
        tiles = c["tiles"]
        t0 = 0
        while t0 < tiles:
            nch = min(SBW, tiles - t0)
            supers.append(dict(cls=c, t0=t0, nch=nch))
            blocks.append((c, t0, nch))
            t0 += nch

    # edge -> (class, tile, row-in-tile) mapping
    eslot = dev * NS + slot_of[dev, ldst]
    order = np.argsort(eslot, kind="stable")
    es = eslot[order]
    gr = grow[order]
    uniq, start_idx, counts = np.unique(es, return_index=True, return_counts=True)
    pos = np.arange(len(es)) - np.repeat(start_idx, counts)
    e_dev = (es // NS).astype(np.int64)
    e_slot = (es % NS).astype(np.int64)

    cls_of_slot = np.zeros(NS, dtype=np.int64)
    for ci, c in enumerate(classes):
        cls_of_slot[c["slot0"]: c["slot0"] + c["tiles"] * c["m"]] = ci
    cK = np.array([c["K"] for c in classes])
    cm = np.array([c["m"] for c in classes])
    cs0 = np.array([c["slot0"] for c in classes])
    ci = cls_of_slot[e_slot]
    rel = e_slot - cs0[ci]
    tt = rel // cm[ci]
    gg = rel % cm[ci]
    rr = gg * cK[ci] + pos
    assert (pos < cK[ci]).all()

    idx_parts = []
    mask_parts = []
    for c, t0, nch in blocks:
        cidx = classes.index(c)
        n_tot = 128 * nch
        valI = np.zeros((NCORE, n_tot), dtype=np.int64)
        valM = np.zeros((NCORE, n_tot), dtype=np.int64)
        msk = (ci == cidx) & (tt >= t0) & (tt < t0 + nch)
        n_e = (tt[msk] - t0) * 128 + rr[msk]
        rowv = gr[msk]
        dv = e_dev[msk]
        valI[dv, n_e] = rowv >> 1
        valM[dv, n_e] = rowv & 1
        # idx wrap int16: n -> [n%16, n//16], replicated over 8 core-groups
        w = valI.reshape(NCORE, n_tot // 16, 16).transpose(0, 2, 1)
        idx_parts.append(np.tile(w, (1, 8, 1)).astype(np.int16))
        # mask layout: position n -> [n%128, n//128], u32 0/0xFFFFFFFF
        mm = valM.reshape(NCORE, nch, 128).transpose(0, 2, 1)  # [C,128,nch]
        mask_parts.append((-mm).astype(np.int64).astype(np.uint32))
    idx1 = np.concatenate(idx_parts, axis=2)
    mask = np.concatenate(mask_parts, axis=2)

    # x rows in slot order (transposed), per device
    def slot_x(d, x):
        xs = np.zeros((NS, FIN), dtype=np.float32)
        valid = node_at[d] >= 0
        xs[valid] = x[d * NPD + node_at[d][valid]]
        return xs

    meta = dict(classes=classes, supers=supers, NS=NS, NSP=NSP, node_at=node_at,
                slot_of=slot_of, idx1=idx1, mask=mask, slot_x=slot_x)
    return meta


def prep_weights(Wl1, Wr1, att1, b1, Wl2, Wr2, att2, b2):
    """Host weight transforms. Returns dict of constant arrays."""
    perm1 = np.argsort(att1 < 0, kind="stable")
    a1p = np.abs(att1[perm1])
    npos1 = int((att1 >= 0).sum())
    wtab_att = np.zeros((FIN, 65), dtype=np.float32)
    wtab_att[:, :64] = Wl1[:, perm1] * a1p[None, :]
    wtab_att[:, 64] = 0.25 * (Wl1 @ att1)
    w_yz = np.concatenate([Wl1 @ Wl2, Wl1 @ Wr2], axis=1).astype(np.float32)  # [128,4]
    wxr = np.zeros((FIN, 65), dtype=np.float32)
    wxr[:, :64] = Wr1[:, perm1] * a1p[None, :]
    wxr[:, 64] = 0.25 * (Wr1 @ att1)

    cy = b1 @ Wl2  # [2]
    cz = b1 @ Wr2
    perm2 = np.argsort(att2 < 0, kind="stable")
    a2p = np.abs(att2[perm2])
    npos2 = int((att2 >= 0).sum())
    return dict(
        wtab_att=wtab_att, w_yz=w_yz, wxr=wxr, npos1=npos1,
        cy=cy, cz=cz, perm2=perm2, a2p=a2p, npos2=npos2,
        att2=att2, b2=b2,
    )


def make_oh_consts(classes):
    """Constant block-one-hot matrices per class (host-built)."""
    ohst = []  # stacked lhsT per class: [m x tile_e] -> [128, tile_e]
    ohb = []   # OH_exd replicated BAT times: [tile_e, BAT*m] -> [128, BAT*m]
    for c in classes:
        K, m, te = c["K"], c["m"], c["tile_e"]
        st = np.zeros((128, te), dtype=np.float32)
        for g in range(m):
            st[g, g * K:(g + 1) * K] = 1.0  # OH_dxe rows
        ohst.append(st)
        ex = np.zeros((te, m), dtype=np.float32)
        for g in range(m):
            ex[g * K:(g + 1) * K, g] = 1.0
        blk = np.zeros((128, BAT * m), dtype=np.float32)
        for j in range(BAT):
            blk[:te, j * m:(j + 1) * m] = ex
        ohb.append(blk)
    return ohst, ohb


# --------------------------------------------------------------------------
# numpy mock of the exact device pipeline (bf16 rounding included)
# --------------------------------------------------------------------------

def mock_run(meta, W, x):
    bf = ml_dtypes.bfloat16
    NS, NSP = meta["NS"], meta["NSP"]
    classes = meta["classes"]

    def bfr(a):
        return a.astype(bf).astype(np.float32)

    # layer-1 table: [NCORE*NSP, 70] (65 logit | 4 yz | one), bf16-rounded
    tables = []
    xr_rows_all = []
    for d in range(NCORE):
        xs = bfr(meta["slot_x"](d, x))
        tab = np.zeros((NSP, 70), dtype=np.float32)
        tab[:NS, 0:65] = bfr(xs @ bfr(W["wtab_att"]))
        tab[:NS, 65:69] = bfr(xs @ bfr(W["w_yz"]))
        tab[:NS, 69] = 1.0
        tab[0] = 0.0
        tables.append(tab)
        xr_r = np.zeros((NSP, 65), np.float32)
        xr_r[:NS] = bfr(xs @ bfr(W["wxr"]))
        xr_rows_all.append(xr_r)
    table_all = np.concatenate(tables, axis=0)

    def unwrap_idx(idx16, n_tot, off16):
        w = idx16[:, off16:off16 + n_tot // 16]
        return w[:16].T.reshape(-1).astype(np.int64)

    def unwrap_mask(mk, n_tot, offm):
        m = mk[:, offm:offm + n_tot // 128]
        return (m.T.reshape(-1).astype(np.int64) & 1)

    def edge_pass(tabL, xrL, NP_, NF, NV):
        # tabL: [NCORE*NSP, NLOG+NV]; logit cols 0:NF+1, value cols NF+1:
        NLOG = NF + 1
        aggs = [np.zeros((NSP, NV), np.float32) for _ in range(NCORE)]
        for d in range(NCORE):
            off16 = 0
            offm = 0
            for c in classes:
                K, m, te, tiles = c["K"], c["m"], c["tile_e"], c["tiles"]
                t0 = 0
                while t0 < tiles:
                    nch = min(SBW, tiles - t0)
                    n_tot = 128 * nch
                    iP = unwrap_idx(meta["idx1"][d], n_tot, off16)
                    mP = unwrap_mask(meta["mask"][d], n_tot, offm)
                    rows = tabL[2 * iP + mP].reshape(nch, 128, -1)
                    g_of = np.arange(te) // K
                    xr_t = np.stack([xrL[d][c["slot0"] + (t0 + ch) * m:
                                            c["slot0"] + (t0 + ch + 1) * m]
                                     for ch in range(nch)], axis=0)
                    H = rows[:, :te, 0:NLOG] + xr_t[:, g_of, 0:NLOG]
                    relu = bfr(np.maximum(H[:, :, 0:NF], 0.0))
                    dpos = relu[:, :, :NP_].sum(2) - relu[:, :, NP_:].sum(2)
                    sv = bfr(np.exp(0.8 * (dpos + H[:, :, NF])))
                    vals = rows[:, :te, NLOG:NLOG + NV]
                    contrib = sv[:, :, None] * vals
                    for ch in range(nch):
                        sl = c["slot0"] + (t0 + ch) * m
                        for g in range(m):
                            aggs[d][sl + g] += contrib[ch, g * K:(g + 1) * K].sum(0)
                    off16 += n_tot // 16
                    offm += nch
                    t0 += nch
        return aggs

    aggs = edge_pass(table_all, xr_rows_all, W["npos1"], 64, 5)
    l2_tabs, xr2_all = [], []
    for d in range(NCORE):
        rcp = 1.0 / (aggs[d][:, 4] + 1e-30)
        sc = aggs[d][:, :4] * rcp[:, None]
        xl2 = sc[:, 0:2] + W["cy"][None, :]
        xr2v = sc[:, 2:4] + W["cz"][None, :]
        p2 = 0.25 * (xl2 @ W["att2"])
        q2 = 0.25 * (xr2v @ W["att2"])
        l2 = np.zeros((NSP, 6), np.float32)
        l2[:, 0:2] = bfr(xl2[:, W["perm2"]] * W["a2p"][None, :])
        l2[:, 2] = bfr(p2)
        l2[:, 3:5] = bfr(xl2)
        l2[:, 5] = 1.0
        l2[0] = 0.0
        l2_tabs.append(l2)
        xr2 = np.zeros((NSP, 3), np.float32)
        xr2[:, 0:2] = bfr(xr2v[:, W["perm2"]] * W["a2p"][None, :])
        xr2[:, 2] = bfr(q2)
        xr2_all.append(xr2)
    l2_all = np.concatenate(l2_tabs, axis=0)

    aggs2 = edge_pass(l2_all, xr2_all, W["npos2"], 2, 3)
    out = np.zeros((N, FOUT), dtype=np.float32)
    for d in range(NCORE):
        rcp = 1.0 / (aggs2[d][:, 2] + 1e-30)
        o2 = aggs2[d][:, :2] * rcp[:, None] + W["b2"][None, :]
        valid = meta["node_at"][d] >= 0
        out[d * NPD + meta["node_at"][d][valid]] = o2[:NS][valid]
    return out


# --------------------------------------------------------------------------
# device program (Bass/Tile)
# --------------------------------------------------------------------------

import concourse.bass as bass
import concourse.bacc as bacc_mod
import concourse.mybir as mybir
import concourse.tile as tile
from concourse.bass_utils import run_bass_kernel_spmd


F32 = mybir.dt.float32
BF16 = mybir.dt.bfloat16
U16 = mybir.dt.uint16
U32 = mybir.dt.uint32
I16 = mybir.dt.int16
AF = mybir.ActivationFunctionType
ALU = mybir.AluOpType
AX = mybir.AxisListType


def _ceil(a, b):
    return -(-a // b)


def build_program(meta, W):
    classes = meta["classes"]
    NS = meta["NS"]
    NSP = _ceil(NS, 128) * 128          # padded slot count
    HPAIR = NCORE * NSP // 2            # pair rows in the gathered tables
    npos1 = W["npos1"]
    nneg1 = 64 - npos1
    npos2 = W["npos2"]
    nneg2 = 2 - npos2
    perm2 = W["perm2"]
    a2p = W["a2p"]
    cy, cz = W["cy"], W["cz"]
    att2, b2 = W["att2"], W["b2"]

    ohst_np, ohb_np = make_oh_consts(classes)
    OHST_W = sum(a.shape[1] for a in ohst_np)
    OHB_W = sum(a.shape[1] for a in ohb_np)

    TOT = NCORE * NSP
    nc = bacc_mod.Bacc(None)
    xT = nc.declare_dram_parameter("xT", [FIN, TOT], BF16, isOutput=False)
    xTown = nc.declare_dram_parameter("xTown", [FIN, NSP], BF16, isOutput=False)
    wtab = nc.declare_dram_parameter("wtab", [FIN, 65], BF16, isOutput=False)
    wyz = nc.declare_dram_parameter("wyz", [FIN, 4], BF16, isOutput=False)
    wxr = nc.declare_dram_parameter("wxr", [FIN, 65], BF16, isOutput=False)
    IDXW = meta["idx1"].shape[2]
    MSKW = meta["mask"].shape[2]
    idx1p = nc.declare_dram_parameter("idx1", [128, IDXW], I16, isOutput=False)
    maskp = nc.declare_dram_parameter("mask", [128, MSKW], U32, isOutput=False)
    eyep = nc.declare_dram_parameter("eye", [128, 128], BF16, isOutput=False)
    ohstp = nc.declare_dram_parameter("ohst", [128, OHST_W], BF16, isOutput=False)
    ohbp = nc.declare_dram_parameter("ohb", [128, OHB_W], BF16, isOutput=False)
    out2 = nc.declare_dram_parameter("out2", [NSP, 2], F32, isOutput=True)

    with tile.TileContext(nc) as tc:
        with (
            tc.tile_pool(name="dram", bufs=1, space="DRAM") as dram,
            tc.tile_pool(name="cpool", bufs=1) as cpool,
            tc.tile_pool(name="sb", bufs=int(os.environ.get("GAT_SBB", 3))) as sb,
            tc.tile_pool(name="sb2", bufs=int(os.environ.get("GAT_SB2", 3))) as sb2,
            tc.tile_pool(name="ps", bufs=int(os.environ.get("GAT_PSB", 5)), space="PSUM") as ps,
            tc.tile_pool(name="psS", bufs=int(os.environ.get("GAT_PSS", 3)), space="PSUM") as psSp,
        ):
            # AG2 split row: end of the first class whose cumulative rows
            # reach ~80% of NS (chunk A all-gathers + expands early, under
            # the tail of edge pass 1)
            rA = NSP
            splitc = len(classes) - 1
            for i, c in enumerate(classes):
                end = c["slot0"] + c["tiles"] * c["m"]
                if end >= 0.8 * NS:
                    rA, splitc = end, i
                    break

            table = dram.tile([HPAIR, PAIRW], U16)
            tabN = table.rearrange("q (t w) -> (q t) w", t=2)
            l2slice = dram.tile([NSP, ROW2_U16], U16)
            l2compA = dram.tile([NCORE * rA, ROW2_U16], U16, addr_space="Shared")
            l2compB = dram.tile([NCORE * (NSP - rA), ROW2_U16], U16,
                                addr_space="Shared")
            l2fat = dram.tile([HPAIR, PAIRW], U16)
            l2fatD = l2fat.rearrange("(d r) (t w) -> d (r t) w", d=NCORE, t=2)
            xr_dram = dram.tile([NSP, XR1_U16], U16)
            xr2_dram = dram.tile([NSP, XR2_U16], U16)

            # ---------------- consts ----------------
            wtab_sb = cpool.tile([FIN, 65], BF16)
            nc.sync.dma_start(wtab_sb[:, :], wtab[:, :])
            wyz_sb = cpool.tile([FIN, 4], BF16)
            nc.sync.dma_start(wyz_sb[:, :], wyz[:, :])
            wxr_sb = cpool.tile([FIN, 65], BF16)
            nc.sync.dma_start(wxr_sb[:, :], wxr[:, :])
            ohst_sb = cpool.tile([128, OHST_W], BF16)
            nc.sync.dma_start(ohst_sb[:, :], ohstp[:, :])
            ohb_sb = cpool.tile([128, OHB_W], BF16)
            nc.sync.dma_start(ohb_sb[:, :], ohbp[:, :])
            eye_sb = cpool.tile([128, 128], BF16)
            nc.sync.dma_start(eye_sb[:, :], eyep[:, :])
            idx_sb = cpool.tile([128, IDXW], I16)
            nc.sync.dma_start(idx_sb[:, :], idx1p[:, :])
            msk_sb = cpool.tile([128, MSKW], U32)
            nc.sync.dma_start(msk_sb[:, :], maskp[:, :])

            # ------- dense phase: full table computed locally (replicated),
            # plus own-slice xr rows -------
            DG = 6       # chunks per psum round
            GW = 12      # chunks per load/store macro-group
            assert (TOT // 128) % GW == 0 and (NSP // 128) % 3 == 0
            for c0 in range(0, TOT // 128, GW):
                xch = sb.tile([128, GW * 128], BF16, tag="xch")
                nc.sync.dma_start(xch[:, :], xT[:, c0 * 128:(c0 + GW) * 128])
                rows = sb.tile([128, GW * ROW1_U16], U16, tag="rows")
                rv = rows.bitcast(BF16).rearrange("p (g w) -> p g w", w=ROW1_U16)
                for r in range(GW // DG):
                    ps_d = ps.tile([128, BAT * 65], F32, tag="psH")
                    pdv = ps_d[:, 0:DG * 69].rearrange("p (g w) -> p g w", w=69)
                    for g in range(DG):
                        gg = r * DG + g
                        lh = xch[:, gg * 128:(gg + 1) * 128]
                        nc.tensor.matmul(out=pdv[:, g, 0:65], lhsT=lh,
                                         rhs=wtab_sb[:, :], start=True, stop=True)
                        nc.tensor.matmul(out=pdv[:, g, 65:69], lhsT=lh,
                                         rhs=wyz_sb[:, :], start=True, stop=True)
                    nc.scalar.activation(rv[:, r * DG:(r + 1) * DG, 0:69],
                                         pdv[:, :, 0:69], AF.Copy)
                nc.vector.memset(rv[:, :, 69:70], 1.0)
                nc.sync.dma_start(
                    tabN[c0 * 128:(c0 + GW) * 128, 0:70]
                    .rearrange("(g p) w -> p g w", p=128),
                    rows[:, :].rearrange("p (g w) -> p g w", w=ROW1_U16)[:, :, 0:70])
            for c0 in range(0, NSP // 128, 3):
                xch = sb.tile([128, 3 * 128], BF16, tag="xcho")
                nc.scalar.dma_start(xch[:, :], xTown[:, c0 * 128:(c0 + 3) * 128])
                ps_x = ps.tile([128, BAT * 65], F32, tag="psH")
                pxv = ps_x[:, 0:3 * 65].rearrange("p (g w) -> p g w", w=65)
                for g in range(3):
                    nc.tensor.matmul(out=pxv[:, g, :],
                                     lhsT=xch[:, g * 128:(g + 1) * 128],
                                     rhs=wxr_sb[:, :], start=True, stop=True)
                xrr = sb.tile([128, 3 * XR1_U16], U16, tag="xrr")
                xv = xrr.bitcast(BF16).rearrange("p (g w) -> p g w", w=XR1_U16)
                nc.scalar.activation(xv[:, :, 0:65], pxv[:, :, :], AF.Copy)
                nc.vector.memset(xv[:, :, 65:66], 0.0)
                nc.scalar.dma_start(
                    xr_dram[c0 * 128:(c0 + 3) * 128, :]
                    .rearrange("(g p) w -> p g w", p=128),
                    xrr.rearrange("p (g w) -> p g w", w=XR1_U16))

            # zero row 0 of the table (the reserved all-zero row)
            zr = sb.tile([1, ROW1_U16], U16, tag="zr")
            nc.vector.memset(zr[:, :], 0)
            nc.sync.dma_start(tabN[0:1, :], zr[:, :])

            # zero row 0 of l2slice before edge pass 1 starts writing it
            zr2 = sb.tile([1, ROW2_U16], U16, tag="zr2")
            nc.vector.memset(zr2[:, :], 0)
            nc.sync.dma_start(l2slice[0:1, :], zr2[:, :])

            def ag2_chunk(which):
                if which == 0:
                    r0, r1, comp = 0, rA, l2compA
                else:
                    r0, r1, comp = rA, NSP, l2compB
                nc.gpsimd.collective_compute(
                    "AllGather", ALU.bypass,
                    replica_groups=[list(range(NCORE))],
                    ins=[l2slice[r0:r1, :]], outs=[comp[:, :]],
                )
                nc.sync.dma_start(
                    l2fatD[:, r0:r1, 0:ROW2_U16],
                    comp.rearrange("(d r) w -> d r w", d=NCORE))

            # ---------------- edge pass helper ----------------
            def edge_pass(lay):
                if lay == 1:
                    tabT, xrT = table, xr_dram
                    SW, VC, XRW = SEL1, VC1, XR1_U16
                    NP_, NN_, NF = npos1, nneg1, 64
                    NV = 5
                else:
                    tabT, xrT = l2fat, xr2_dram
                    SW, VC, XRW = SEL2, VC2, XR2_U16
                    NP_, NN_, NF = npos2, nneg2, 2
                    NV = 3
                NLOG = NF + 1  # logit cols incl p/q col
                idx_off = 0
                msk_off = 0
                ohst_off = 0
                ohb_off = 0
                for cls_i, cls in enumerate(classes):
                    K, m, te, tiles, slot0 = (cls["K"], cls["m"], cls["tile_e"],
                                              cls["tiles"], cls["slot0"])
                    psS = None
                    f_t0 = 0

                    def flush(ntl):
                        # flush tiles [f_t0, f_t0+ntl) of this class
                        P = psS[:, 0:FT * NV].rearrange("p (j v) -> p j v", v=NV)
                        rcpi = sb.tile([128, FT], F32, tag="rcpi")
                        nc.vector.tensor_scalar(
                            out=rcpi[0:m, 0:ntl], in0=P[0:m, 0:ntl, NV - 1],
                            scalar1=1e-30, scalar2=None, op0=ALU.add)
                        rcp = sb.tile([128, FT], F32, tag="rcp")
                        nc.vector.reciprocal(rcp[0:m, 0:ntl], rcpi[0:m, 0:ntl])
                        sc = sb.tile([128, FT * 4], F32, tag="sc")
                        scv = sc.rearrange("p (j v) -> p j v", v=4)
                        nc.vector.tensor_tensor(
                            out=scv[0:m, 0:ntl, 0:NV - 1], in0=P[0:m, 0:ntl, 0:NV - 1],
                            in1=rcp[0:m, 0:ntl].rearrange("p (j o) -> p j o", o=1)
                                .to_broadcast([m, ntl, NV - 1]),
                            op=ALU.mult)
                        r0 = slot0 + f_t0 * m
                        nrows = ntl * m
                        if lay == 1:
                            l2r = sb.tile([128, FT * ROW2_U16], U16, tag="l2r")
                            lb = l2r.bitcast(BF16).rearrange(
                                "p (j v) -> p j v", v=ROW2_U16)
                            x2r = sb.tile([128, FT * XR2_U16], U16, tag="x2r")
                            xb = x2r.bitcast(BF16).rearrange(
                                "p (j v) -> p j v", v=XR2_U16)
                            t1 = sb.tile([128, FT], F32, tag="t1")
                            t2 = sb.tile([128, FT], F32, tag="t2")
                            for cc in range(2):
                                nc.vector.tensor_scalar(
                                    out=lb[0:m, 0:ntl, cc], in0=scv[0:m, 0:ntl, perm2[cc]],
                                    scalar1=float(a2p[cc]),
                                    scalar2=float(a2p[cc] * cy[perm2[cc]]),
                                    op0=ALU.mult, op1=ALU.add)
                                nc.vector.tensor_scalar(
                                    out=xb[0:m, 0:ntl, cc], in0=scv[0:m, 0:ntl, 2 + perm2[cc]],
                                    scalar1=float(a2p[cc]),
                                    scalar2=float(a2p[cc] * cz[perm2[cc]]),
                                    op0=ALU.mult, op1=ALU.add)
                            nc.vector.tensor_scalar(
                                out=t1[0:m, 0:ntl], in0=scv[0:m, 0:ntl, 0],
                                scalar1=float(0.25 * att2[0]),
                                scalar2=float(0.25 * (att2 @ cy)),
                                op0=ALU.mult, op1=ALU.add)
                            nc.vector.tensor_scalar(
                                out=t2[0:m, 0:ntl], in0=scv[0:m, 0:ntl, 1],
                                scalar1=float(0.25 * att2[1]), scalar2=None, op0=ALU.mult)
                            nc.vector.tensor_tensor(
                                out=lb[0:m, 0:ntl, 2], in0=t1[0:m, 0:ntl],
                                in1=t2[0:m, 0:ntl], op=ALU.add)
                            nc.vector.tensor_scalar(
                                out=t1[0:m, 0:ntl], in0=scv[0:m, 0:ntl, 2],
                                scalar1=float(0.25 * att2[0]),
                                scalar2=float(0.25 * (att2 @ cz)),
                                op0=ALU.mult, op1=ALU.add)
                            nc.vector.tensor_scalar(
                                out=t2[0:m, 0:ntl], in0=scv[0:m, 0:ntl, 3],
                                scalar1=float(0.25 * att2[1]), scalar2=None, op0=ALU.mult)
                            nc.vector.tensor_tensor(
                                out=xb[0:m, 0:ntl, 2], in0=t1[0:m, 0:ntl],
                                in1=t2[0:m, 0:ntl], op=ALU.add)
                            nc.vector.memset(xb[0:m, 0:ntl, 3], 0.0)
                            for cc in range(2):
                                nc.vector.tensor_scalar(
                                    out=lb[0:m, 0:ntl, 3 + cc],
                                    in0=scv[0:m, 0:ntl, cc],
                                    scalar1=float(cy[cc]), scalar2=None, op0=ALU.add)
                            nc.vector.memset(lb[0:m, 0:ntl, 5], 1.0)
                            for (buf, dstt, w) in ((l2r, l2slice, ROW2_U16),
                                                   (x2r, xr2_dram, XR2_U16)):
                                dst_ap = dstt[r0:r0 + nrows, 0:w] \
                                    .rearrange("(j p) w -> p j w", p=m)
                                src_ap = buf.rearrange("p (j v) -> p j v", v=w)[
                                    0:m, 0:ntl, :]
                                nc.sync.dma_start(dst_ap, src_ap)
                        else:
                            o2 = sb.tile([128, FT * 2], F32, tag="o2")
                            o2v = o2.rearrange("p (j v) -> p j v", v=2)
                            for cc in range(2):
                                nc.vector.tensor_scalar(
                                    out=o2v[0:m, 0:ntl, cc], in0=scv[0:m, 0:ntl, cc],
                                    scalar1=float(b2[cc]), scalar2=None, op0=ALU.add)
                            dst_ap = out2[r0:r0 + nrows, :] \
                                .rearrange("(j p) w -> p j w", p=m)
                            nc.sync.dma_start(dst_ap, o2v[0:m, 0:ntl, :])

                    t0 = 0
                    while t0 < tiles:
                        nch = min(SBW, tiles - t0)
                        STP = sb2.tile([128, SBW * PAIRW], U16, tag="STP")
                        c16 = idx_off // 16
                        nc.gpsimd.dma_gather(
                            out_ap=STP[:, 0:nch * PAIRW]
                            .rearrange("p (k w) -> p k w", w=PAIRW),
                            in_ap=tabT[0:HPAIR, :],
                            idxs_ap=idx_sb[:, c16:c16 + 8 * nch],
                            num_idxs=128 * nch, num_idxs_reg=128 * nch,
                            elem_size=PAIRW, single_packet=False)
                        # pair select: STS = lo ^ ((lo ^ hi) & mask), u32 ALU
                        SW2 = SW // 2
                        STP32 = STP.bitcast(U32).rearrange(
                            "p (k w) -> p k w", w=PAIRW // 2)
                        STS = sb2.tile([128, SBW * SW], U16, tag="STS")
                        sv_ = STS.bitcast(U32).rearrange("p (k w) -> p k w", w=SW2)
                        nc.vector.tensor_tensor(
                            out=sv_[:, 0:nch, :], in0=STP32[:, 0:nch, 0:SW2],
                            in1=STP32[:, 0:nch, 64:64 + SW2], op=ALU.bitwise_xor)
                        nc.vector.tensor_tensor(
                            out=sv_[:, 0:nch, :], in0=sv_[:, 0:nch, :],
                            in1=msk_sb[:, msk_off:msk_off + nch]
                            .rearrange("p (k o) -> p k o", o=1)
                            .to_broadcast([128, nch, SW2]),
                            op=ALU.bitwise_and)
                        nc.vector.tensor_tensor(
                            out=sv_[:, 0:nch, :], in0=sv_[:, 0:nch, :],
                            in1=STP32[:, 0:nch, 0:SW2], op=ALU.bitwise_xor)
                        STSb = STS.bitcast(BF16).rearrange("p (k w) -> p k w", w=SW)
                        # xr rows for these tiles
                        xrst = sb2.tile([128, SBW * XR1_U16], U16, tag="xrst")
                        xru = xrst.rearrange("p (k w) -> p k w", w=XR1_U16)
                        r0 = slot0 + t0 * m
                        nc.scalar.dma_start(
                            xru[0:m, 0:nch, 0:XRW],
                            xrT[r0:r0 + nch * m, 0:XRW]
                            .rearrange("(c g) w -> g c w", g=m))
                        xrb = xrst.bitcast(BF16).rearrange("p (k w) -> p k w", w=XR1_U16)
                        for b in range(_ceil(nch, BAT)):
                            nb = min(BAT, nch - b * BAT)
                            bs = slice(b * BAT, b * BAT + nb)
                            psH = ps.tile([128, BAT * NLOG], F32, tag="psH")
                            pHv = psH.rearrange("p (b w) -> p b w", w=NLOG)
                            nc.tensor.matmul(
                                out=pHv[0:te, 0:nb, :],
                                lhsT=eye_sb[0:te, 0:te],
                                rhs=STSb[0:te, bs, 0:NLOG],
                                start=True, stop=False)
                            nc.tensor.matmul(
                                out=pHv[0:te, 0:nb, :],
                                lhsT=ohst_sb[0:m, ohst_off:ohst_off + te],
                                rhs=xrb[0:m, bs, 0:NLOG],
                                start=False, stop=True)
                            Hr = sb.tile([128, BAT * NF], BF16, tag=f"Hr{lay}")
                            Hv = Hr.rearrange("p (b w) -> p b w", w=NF)
                            nc.scalar.activation(
                                Hv[0:te, 0:nb, :], pHv[0:te, 0:nb, 0:NF], AF.Relu)
                            dt = sb.tile([128, BAT], F32, tag=f"dt{lay}")
                            if NP_ > 0 and NN_ > 0:
                                Ap = sb.tile([128, BAT], F32, tag=f"Ap{lay}")
                                An = sb.tile([128, BAT], F32, tag=f"An{lay}")
                                nc.vector.tensor_reduce(
                                    out=Ap[0:te, 0:nb], in_=Hv[0:te, 0:nb, 0:NP_],
                                    axis=AX.X, op=ALU.add)
                                nc.vector.tensor_reduce(
                                    out=An[0:te, 0:nb], in_=Hv[0:te, 0:nb, NP_:NF],
                                    axis=AX.X, op=ALU.add)
                                nc.vector.tensor_tensor(
                                    out=dt[0:te, 0:nb], in0=Ap[0:te, 0:nb],
                                    in1=An[0:te, 0:nb], op=ALU.subtract)
                            else:
                                nc.vector.tensor_reduce(
                                    out=dt[0:te, 0:nb], in_=Hv[0:te, 0:nb, 0:NF],
                                    axis=AX.X, op=ALU.add)
                                if NN_ > 0:
                                    nc.vector.tensor_scalar(
                                        out=dt[0:te, 0:nb], in0=dt[0:te, 0:nb],
                                        scalar1=-1.0, scalar2=None, op0=ALU.mult)
                            ep = sb.tile([128, BAT], F32, tag=f"ep{lay}")
                            nc.vector.tensor_tensor(
                                out=ep[0:te, 0:nb], in0=dt[0:te, 0:nb],
                                in1=pHv[0:te, 0:nb, NF], op=ALU.add)
                            sB = sb.tile([128, BAT], F32, tag=f"sB{lay}")
                            nc.scalar.activation(
                                sB[0:te, 0:nb], ep[0:te, 0:nb], AF.Exp, scale=0.8)
                            soh = sb.tile([128, BAT * 33], BF16, tag=f"soh{lay}")
                            sohv = soh.rearrange("p (b w) -> p b w", w=33)
                            nc.vector.tensor_tensor(
                                out=sohv[0:te, 0:nb, 0:m],
                                in0=ohb_sb[0:te, ohb_off:ohb_off + nb * m]
                                .rearrange("p (b w) -> p b w", w=m),
                                in1=sB[0:te, 0:nb]
                                .rearrange("p (b o) -> p b o", o=1)
                                .to_broadcast([te, nb, m]),
                                op=ALU.mult)
                            for j in range(nb):
                                tg = t0 + b * BAT + j
                                jj = tg - f_t0
                                if jj == 0:
                                    psS = psSp.tile([128, FT * NV], F32,
                                                    tag="psS")
                                kabs = b * BAT + j
                                nc.tensor.matmul(
                                    out=psS[0:m, jj * NV:(jj + 1) * NV],
                                    lhsT=sohv[0:te, j, 0:m],
                                    rhs=STSb[0:te, kabs, VC:VC + NV],
                                    start=True, stop=True)
                                if jj == FT - 1 or tg == tiles - 1:
                                    flush(jj + 1)
                                    f_t0 = tg + 1
                                    psS = None
                        idx_off += 128 * nch
                        msk_off += nch
                        t0 += nch
                    ohst_off += te
                    ohb_off += BAT * m
                    if lay == 1 and cls_i == splitc:
                        ag2_chunk(0)

            edge_pass(1)
            ag2_chunk(1)
            edge_pass(2)

    return nc, NSP


def run_device(meta, W, x, trace=False):
    nc, NSP = build_program(meta, W)
    NS = meta["NS"]
    assert NSP == meta["NSP"]
    classes = meta["classes"]
    ohst_np, ohb_np = make_oh_consts(classes)
    ohst = np.concatenate(ohst_np, axis=1).astype(ml_dtypes.bfloat16)
    ohb = np.concatenate(ohb_np, axis=1).astype(ml_dtypes.bfloat16)

    bf = ml_dtypes.bfloat16
    slices = []
    for d in range(NCORE):
        xsp = np.zeros((NSP, FIN), dtype=np.float32)
        xsp[:NS] = meta["slot_x"](d, x)
        slices.append(xsp)
    xall_T = np.ascontiguousarray(
        np.concatenate(slices, axis=0).T).astype(bf)  # [FIN, NCORE*NSP]

    in_maps = []
    for d in range(NCORE):
        im = dict(
            xT=xall_T,
            xTown=np.ascontiguousarray(slices[d].T).astype(bf),
            wtab=W["wtab_att"].astype(bf),
            wyz=W["w_yz"].astype(bf),
            wxr=W["wxr"].astype(bf),
            idx1=meta["idx1"][d],
            mask=meta["mask"][d],
            eye=np.eye(128, dtype=bf),
            ohst=ohst,
            ohb=ohb,
        )
        in_maps.append(im)

    if not nc.is_finalized():
        nc.finalize()
    res = run_bass_kernel_spmd(nc, in_maps, list(range(NCORE)), trace=trace)
    outs = res.results
    out = np.zeros((N, FOUT), dtype=np.float32)
    for d in range(NCORE):
        o = outs[d]["out2"]
        valid = meta["node_at"][d] >= 0
        out[d * NPD + meta["node_at"][d][valid]] = o[:NS][valid]
    return out, res


# --------------------------------------------------------------------------
# entry
# --------------------------------------------------------------------------

def kernel(**inputs):
    x = np.asarray(inputs["x"], dtype=np.float32)
    meta = build_schedule(np.asarray(inputs["edge_index"]))
    W = prep_weights(
        np.asarray(inputs["Wl1"], np.float32), np.asarray(inputs["Wr1"], np.float32),
        np.asarray(inputs["att1"], np.float32), np.asarray(inputs["b1"], np.float32),
        np.asarray(inputs["Wl2"], np.float32), np.asarray(inputs["Wr2"], np.float32),
        np.asarray(inputs["att2"], np.float32), np.asarray(inputs["b2"], np.float32),
    )
    if os.environ.get("GAT_MOCK"):
        return mock_run(meta, W, x)
    out, _res = run_device(meta, W, x)
    return out


if __name__ == "__main__":
    pass
